# revision 50
# baseline (speedup 1.0000x reference)
"""Causal self-attention (B=2,T=2048,C=1024,H=16) on 8 trn2 cores.

Sharding: core = (batch b, head-group g); b = core//4, g = core%4.
Each core computes attention for 4 heads of one batch plus the
row-parallel slice of c_proj; host sums the 4 partial projections per
batch and adds b_proj.

v2: bf16 matmul path (tolerance 2e-2 allows it), chunk-interleaved
input DMA overlapped with the QKV matmuls, head-pair row-tiled score
matmuls, diagonal tiles padded to N>=256, c_proj DMA'd straight from
PSUM, softmax denominator broadcast into the spare partitions of the
attention PSUM bank.
"""

import numpy as np
from contextlib import ExitStack

import concourse.bass as bass
import concourse.mybir as mybir
import concourse.tile as tile
from concourse.bass import ts, ds
from concourse.bass_utils import run_bass_kernel_spmd
from concourse.vector_clock import ScopedClock

# ---------------------------------------------------------------------------
# Workaround: walrus CoreV3 rejects >2 sem waits on one instruction; the
# TileContext exit drain accumulates one wait per outstanding proc.  Split
# them across single-wait sync nops before the drain.
# ---------------------------------------------------------------------------


def _patched_drain_and_barrier(self, tick_clock, wait_clock):
    nc = self.nc
    probe = mybir.InstNoOp(name=nc.get_next_instruction_name(), ins=[], outs=[])
    probe.engine = mybir.EngineType.SP
    wait_clock.add_sem_waits(probe, ScopedClock({None: tick_clock.global_clock}))
    waits = list(probe.sync_info.on_wait) if probe.sync_info else []
    for w in waits:
        n = nc.sync.nop(nofuse=True, hint="drain_wait_split")
        n.ins.sync_info = mybir.SyncInfo(on_wait=[w], on_update=[])
    nc.sync.drain()
    nc.all_engine_barrier()
    assert self.sems is not None
    popped = nc._tile_sem_poison_stack.pop()
    assert popped is self._sem_poison
    nc.clear_and_free_semaphores(list(self.sems.allocated().values()))
    nc.all_engine_barrier()


tile.TileContext._drain_and_barrier = _patched_drain_and_barrier

_DMA_INSTS = (
    mybir.InstCollectiveCompute,
)


def split_excess_waits(nc):
    """walrus CoreV3 encodes at most 1 sem wait per compute instruction
    (2 on EventSemaphore); hoist extras onto same-engine nops."""
    for fn in nc.m.functions:
        for bb in fn.blocks:
            insts = bb.instructions
            new_list = []
            changed = False
            for inst in insts:
                si = inst.sync_info
                cap = 2 if isinstance(inst, mybir.InstEventSemaphore) else 1
                if (
                    si is not None
                    and not isinstance(inst, _DMA_INSTS)
                    and len(si.on_wait) > cap
                ):
                    waits = list(si.on_wait)
                    extra, keep = waits[:-cap], waits[-cap:]
                    for w in extra:
                        nop = mybir.InstNoOp(
                            name=nc.get_next_instruction_name(), ins=[], outs=[]
                        )
                        nop.engine = inst.engine
                        nop.sync_info = mybir.SyncInfo(on_wait=[w], on_update=[])
                        nc.register_instruction(nop)
                        new_list.append(nop)
                    inst.sync_info = mybir.SyncInfo(
                        on_wait=keep, on_update=list(si.on_update)
                    )
                    changed = True
                new_list.append(inst)
            if changed:
                bb.instructions = new_list

# ---------------------------------------------------------------------------

B, T, C, H, HD = 2, 2048, 1024, 16, 64
NCORES, GROUPS = 8, 4
CL = C // GROUPS          # 256 channels (4 heads) per core
HPC = H // GROUPS         # 4 heads per core
F32 = mybir.dt.float32
R32 = mybir.dt.float32r
BF16 = mybir.dt.bfloat16

QT = 512                  # q window (free dim of S^T tiles)
NQW = T // QT             # 4 q windows
NKT = T // 128            # 16 k tiles of 128

import os
REPS = int(os.environ.get("KREPS", "1"))
SECTIONS = os.environ.get("KSECTIONS", "full")  # qkv | qkvattn | full


def build_nc():
    nc = bass.Bass()
    xT = nc.dram_tensor("xT", [C, T], BF16, kind="ExternalInput")
    wqkT = nc.dram_tensor("wqkT", [C, 2 * CL], BF16, kind="ExternalInput")
    wvT = nc.dram_tensor("wvT", [C, CL], BF16, kind="ExternalInput")
    wpT = nc.dram_tensor("wpT", [CL, C], BF16, kind="ExternalInput")
    bqk = nc.dram_tensor("bqk", [2 * CL], F32, kind="ExternalInput")
    bvb = nc.dram_tensor("bvb", [128, CL], F32, kind="ExternalInput")
    m0 = nc.dram_tensor("m0", [128, 128], BF16, kind="ExternalInput")
    ones = nc.dram_tensor("ones", [128, NKT * HPC], BF16, kind="ExternalInput")
    outp = nc.dram_tensor("outp", [T, C], BF16, kind="ExternalOutput")

    AF = mybir.ActivationFunctionType
    OP = mybir.AluOpType

    with tile.TileContext(nc) as tc, ExitStack() as ctx:
        persist = ctx.enter_context(tc.tile_pool(name="persist", bufs=1))
        qkvin = ctx.enter_context(tc.tile_pool(name="qkvin", bufs=1))
        work = ctx.enter_context(tc.tile_pool(name="work", bufs=4))
        bcast = ctx.enter_context(tc.tile_pool(name="bcast", bufs=4))
        outsb = ctx.enter_context(tc.tile_pool(name="outsb", bufs=2))
        psS = ctx.enter_context(tc.tile_pool(name="psS", bufs=2, space="PSUM"))
        psB = ctx.enter_context(tc.tile_pool(name="psB", bufs=4, space="PSUM"))

        # persistent tensors
        qkT = persist.tile([128, 4, T], BF16)         # o-tiles: q01 q23 k01 k23
        vaug = persist.tile([128, NKT, HPC, HD + 1], BF16)
        yT = persist.tile([128, 2, T], BF16)          # heads stacked on (part, chunk)
        wp_s = persist.tile([128, 2, C], BF16)
        m0_s = persist.tile([128, 128], BF16)
        bq_s = persist.tile([128, 4], F32)
        bv_s = persist.tile([128, CL], F32)
        ones_f = persist.tile([1, 64], BF16)

        # small persistent loads on the gpsimd SWDGE queue
        nc.gpsimd.dma_start(out=m0_s, in_=m0[:, :])
        nc.gpsimd.dma_start(out=bq_s, in_=bqk.rearrange("(o p) -> p o", p=128))
        nc.gpsimd.dma_start(out=bv_s, in_=bvb[:, :])
        nc.gpsimd.dma_start(
            out=vaug[:, :, :, HD:HD + 1],
            in_=ones.rearrange("p (a b) -> p a b", b=HPC).unsqueeze(3),
        )
        nc.gpsimd.dma_start(out=ones_f, in_=ones[0:1, 0:64])

        # ---- QKV inputs: chunk-interleaved so matmuls can start early ----
        xTs = qkvin.tile([128, 8, T], BF16)
        wqk_s = qkvin.tile([128, 8, 2 * CL], BF16)
        wv_s = qkvin.tile([128, 8, CL], BF16)
        xT_r = xT.rearrange("(cc p) t -> p cc t", p=128)
        wqk_r = wqkT.rearrange("(cc p) o -> p cc o", p=128)
        wv_r = wvT.rearrange("(cc p) o -> p cc o", p=128)
        # chunk-pair transfers; weights on the scalar HWDGE queue so both
        # queues issue in parallel at kernel start
        for cc in range(0, 8, 2):
            nc.scalar.dma_start(
                out=wqk_s[:, cc:cc + 2, :], in_=wqk_r[:, cc:cc + 2, :]
            )
            nc.sync.dma_start(
                out=xTs[:, cc:cc + 2, ts(0, QT)],
                in_=xT_r[:, cc:cc + 2, ts(0, QT)],
            )
        for cc in range(0, 8, 4):
            nc.scalar.dma_start(
                out=wv_s[:, cc:cc + 4, :], in_=wv_r[:, cc:cc + 4, :]
            )
        for t in range(1, 4):
            for cc in range(0, 8, 2):
                nc.sync.dma_start(
                    out=xTs[:, cc:cc + 2, ts(t, QT)],
                    in_=xT_r[:, cc:cc + 2, ts(t, QT)],
                )
        nc.scalar.dma_start(
            out=wp_s, in_=wpT.rearrange("(cc p) o -> p cc o", p=128)
        )

        def emit_qk(t, part):
            """q^T (part=0) or k^T (part=1) for t-window t: one psS tile
            holding 2 o-accumulators."""
            ps = psS.tile([128, 2, QT], F32, tag="s", name="pqk")
            for cc in range(8):
                for oi in range(2):
                    nc.tensor.matmul(
                        ps[:, oi, :],
                        lhsT=wqk_s[:, cc, ts(2 * part + oi, 128)],
                        rhs=xTs[:, cc, ts(t, QT)],
                        start=(cc == 0),
                        stop=(cc == 7),
                    )
            for oi in range(2):
                o = 2 * part + oi
                nc.vector.tensor_scalar(
                    out=qkT[:, o, ts(t, QT)],
                    in0=ps[:, oi, :],
                    scalar1=0.125 if part == 0 else 1.0,
                    scalar2=bq_s[:, o:o + 1],
                    op0=OP.mult,
                    op1=OP.add,
                )

        def emit_qk01(t):
            """Fused q+k for t-window t, interleaved per chunk so both
            stream behind the input DMA (used for t=0)."""
            pa = psS.tile([128, 2, QT], F32, tag="s", name="pa")
            pb = psS.tile([128, 2, QT], F32, tag="s", name="pb")
            for cc in range(8):
                for oi in range(2):
                    nc.tensor.matmul(
                        pa[:, oi, :],
                        lhsT=wqk_s[:, cc, ts(oi, 128)],
                        rhs=xTs[:, cc, ts(t, QT)],
                        start=(cc == 0),
                        stop=(cc == 7),
                    )
                for oi in range(2):
                    nc.tensor.matmul(
                        pb[:, oi, :],
                        lhsT=wqk_s[:, cc, ts(2 + oi, 128)],
                        rhs=xTs[:, cc, ts(t, QT)],
                        start=(cc == 0),
                        stop=(cc == 7),
                    )
            for o in (0, 2, 1, 3):
                src = pa if o < 2 else pb
                nc.vector.tensor_scalar(
                    out=qkT[:, o, ts(t, QT)],
                    in0=src[:, o % 2, :],
                    scalar1=0.125 if o < 2 else 1.0,
                    scalar2=bq_s[:, o:o + 1],
                    op0=OP.mult,
                    op1=OP.add,
                )

        def emit_v(t):
            for tl in range(4):
                tt = 4 * t + tl
                psv = psB.tile([128, CL], F32, tag="y", name="psv")
                for cc in range(8):
                    nc.tensor.matmul(
                        psv,
                        lhsT=xTs[:, cc, ts(tt, 128)],
                        rhs=wv_s[:, cc, :],
                        start=(cc == 0),
                        stop=(cc == 7),
                    )
                nc.vector.tensor_add(
                    out=vaug[:, tt, :, 0:HD],
                    in0=psv.rearrange("p (h d) -> p h d", h=HPC),
                    in1=bv_s.rearrange("p (h d) -> p h d", h=HPC),
                )

        def emit_proj(j, act_assist=False):
            """c_proj for the 4 t-tiles of window j (yT must be final).
            Uses psB (not psS) so c_proj never stalls the S pipeline.
            act_assist alternates the PSUM->SBUF copies onto ACT (idle in
            the kernel tail)."""
            for tl in range(4):
                tt = j * 4 + tl
                ob = outsb.tile([128, C], BF16, tag="ob")
                for nn_ in range(2):
                    po = psB.tile([128, 512], F32, tag="y", name="po")
                    for c2 in range(2):
                        nc.tensor.matmul(
                            po,
                            lhsT=yT[:, c2, ts(tt, 128)],
                            rhs=wp_s[:, c2, ds(512 * nn_, 512)],
                            start=(c2 == 0),
                            stop=(c2 == 1),
                        )
                    if act_assist and nn_ == 0:
                        nc.scalar.copy(out=ob[:, ts(nn_, 512)], in_=po)
                    else:
                        nc.vector.tensor_copy(out=ob[:, ts(nn_, 512)], in_=po)
                    nc.sync.dma_start(
                        out=outp[ts(tt, 128), ts(nn_, 512)],
                        in_=ob[:, ts(nn_, 512)],
                    )

        def emit_attention(j, mid_hook=None, v_hook=None, hp1_hook=None):
            """Attention for q-window j.  `mid_hook` is emitted inside
            hp=0 right before the diagonal k-tiles: the preceding "old"
            k-tiles only need q(j), so their score matmuls + exps run
            while the hook computes k(j)/v(j) (and the previous window's
            c_proj), keeping ACT fed."""
            nkt = 4 * (j + 1)            # causal: only k tiles <= window end
            for hp in range(2):
                psy = [
                    psB.tile([128, QT], F32, tag="y", name=f"psy{w}")
                    for w in range(2)
                ]
                pending = []
                for kt in range(nkt):
                    m = kt - 4 * j
                    if hp == 0 and m == 0 and mid_hook is not None:
                        mid_hook()
                        mid_hook = None
                    if hp == 0 and m == 2 and v_hook is not None:
                        v_hook()
                        v_hook = None
                    if hp == 1 and kt == 2 and hp1_hook is not None:
                        hp1_hook()
                        hp1_hook = None
                    if m < 0:
                        lo_mm = lo_true = 0
                    else:
                        lo_true = 128 * m
                        lo_mm = min(lo_true, 256)
                    pss = psS.tile([128, 2, QT], F32, tag="s", name="pss")
                    pt = work.tile([128, 2, QT], BF16, tag="pt")
                    for w in range(2):
                        nc.tensor.matmul(
                            pss[:, w, lo_mm:QT],
                            lhsT=qkT[ds(64 * w, 64), 2 + hp, ts(kt, 128)],
                            rhs=qkT[ds(64 * w, 64), hp, ds(j * QT + lo_mm, QT - lo_mm)],
                            start=True,
                            stop=True,
                        )
                    nc.scalar.activation(
                        out=pt[:, :, lo_true:QT],
                        in_=pss[:, :, lo_true:QT],
                        func=AF.Exp,
                    )
                    if lo_mm < lo_true:
                        nc.gpsimd.memset(pt[:, :, lo_mm:lo_true], 0.0)
                    if m >= 0:
                        for w in range(2):
                            nc.gpsimd.tensor_mul(
                                out=pt[:, w, ds(lo_true, 128)],
                                in0=pt[:, w, ds(lo_true, 128)],
                                in1=m0_s,
                            )
                    pending.append((kt, pt, lo_mm))
                    if len(pending) > 3:
                        fkt, fpt, flo = pending.pop(0)
                        for w in range(2):
                            nc.tensor.matmul(
                                psy[w][0:65, flo:QT],
                                lhsT=vaug[:, fkt, 2 * hp + w, :],
                                rhs=fpt[:, w, flo:QT],
                                start=(fkt == 0),
                                stop=(fkt == nkt - 1),
                            )
                for fkt, fpt, flo in pending:
                    for w in range(2):
                        nc.tensor.matmul(
                            psy[w][0:65, flo:QT],
                            lhsT=vaug[:, fkt, 2 * hp + w, :],
                            rhs=fpt[:, w, flo:QT],
                            start=(fkt == 0),
                            stop=(fkt == nkt - 1),
                        )
                # normalize: 1/denom broadcast into rows 64:128 of psy,
                # then staged to SBUF (walrus allows only one PSUM operand
                # per tensor_tensor); op-major order pipelines the two w's
                rcs, dns = [], []
                for w in range(2):
                    rc = bcast.tile([1, QT], BF16, tag="rc", name=f"rc{w}")
                    with nc.allow_low_precision(reason="1/denom broadcast"):
                        nc.vector.reciprocal(out=rc, in_=psy[w][64:65, :])
                    rcs.append(rc)
                for w in range(2):
                    nc.tensor.matmul(
                        psy[w][64:128, :],
                        lhsT=ones_f,
                        rhs=rcs[w],
                        start=True,
                        stop=True,
                    )
                for w in range(2):
                    dn = bcast.tile([64, QT], F32, tag="dn", name=f"dn{w}")
                    nc.vector.tensor_copy(out=dn, in_=psy[w][64:128, :])
                    dns.append(dn)
                for w in range(2):
                    nc.vector.tensor_mul(
                        out=yT[ds(64 * w, 64), hp, ts(j, QT)],
                        in0=psy[w][0:64, :],
                        in1=dns[w],
                    )


        for rep in range(REPS):
            # software-pipelined phases: q(t) -> S/exp for old k-tiles of
            # window t (feeding ACT) -> k(t), v(t), c_proj(t-1) -> rest of
            # window t's attention.  q(t+1) is hoisted between the head
            # pairs so its DVE bias op clears before window t+1 needs it.
            emit_qk01(0)
            for t in range(4):
                if t == 0:
                    hook = None

                    def vhook(tt=t):
                        emit_v(tt)
                else:
                    def hook(tt=t):
                        emit_qk(tt, 1)
                        emit_v(tt)
                        if SECTIONS == "full":
                            emit_proj(tt - 1)

                    vhook = None

                def hp1h(tt=t):
                    if tt + 1 < 4:
                        emit_qk(tt + 1, 0)

                if SECTIONS in ("qkvattn", "full"):
                    emit_attention(
                        t, mid_hook=hook, v_hook=vhook, hp1_hook=hp1h
                    )
                else:
                    if hook is not None:
                        hook()
                    if vhook is not None:
                        vhook()
                    hp1h()
            if SECTIONS == "full":
                emit_proj(3, act_assist=True)

    split_excess_waits(nc)
    return nc


_NC_CACHE = None


def _get_nc():
    global _NC_CACHE
    if _NC_CACHE is None:
        _NC_CACHE = build_nc()
    return _NC_CACHE


def make_in_maps(x, W_attn, b_attn, W_proj):
    import ml_dtypes
    bf16 = ml_dtypes.bfloat16
    x = np.asarray(x, np.float32)
    W_attn = np.asarray(W_attn, np.float32)
    b_attn = np.asarray(b_attn, np.float32)
    W_proj = np.asarray(W_proj, np.float32)
    m0 = np.triu(np.ones((128, 128), np.float32))  # keep q >= k
    in_maps = []
    for core in range(NCORES):
        b, g = core // GROUPS, core % GROUPS
        qr = slice(g * CL, (g + 1) * CL)
        kr = slice(C + g * CL, C + (g + 1) * CL)
        vr = slice(2 * C + g * CL, 2 * C + (g + 1) * CL)
        wqk = np.concatenate([W_attn[qr], W_attn[kr]], axis=0)      # [512, 1024]
        in_maps.append({
            "xT": np.ascontiguousarray(x[b].T).astype(bf16),
            "wqkT": np.ascontiguousarray(wqk.T).astype(bf16),
            "wvT": np.ascontiguousarray(W_attn[vr].T).astype(bf16),
            "wpT": np.ascontiguousarray(W_proj[:, g * CL:(g + 1) * CL].T).astype(bf16),
            "bqk": np.concatenate([b_attn[qr] / 8.0, b_attn[kr]]),
            "bvb": np.broadcast_to(b_attn[vr], (128, CL)).copy(),
            "m0": m0.astype(bf16),
            "ones": np.ones((128, NKT * HPC), bf16),
        })
    return in_maps


def kernel(x, W_attn, b_attn, W_proj, b_proj, **_unused):
    nc = _get_nc()
    in_maps = make_in_maps(x, W_attn, b_attn, W_proj)
    res = run_bass_kernel_spmd(nc, in_maps, core_ids=list(range(NCORES)))
    out = np.zeros((B, T, C), np.float32)
    for core in range(NCORES):
        out[core // GROUPS] += np.asarray(
            res.results[core]["outp"], dtype=np.float32
        )
    out += np.asarray(b_proj, np.float32)[None, None, :]
    return out
